# revision 1
# baseline (speedup 1.0000x reference)
"""AutoCorrelation (Autoformer-style) Bass kernel for one TRN2 chip (8 NeuronCores).

Math: the reference computes, per (b, h):
    corr = irfft(rfft(q, axis=-1) * conj(rfft(k, axis=-1)), n=L)   # [L, L]
    weights = softmax(corr - mean_h(corr), axis=-1)
    Vt = v @ weights                                                # [d, L]
The rfft runs over the d=64 channel axis and the irfft zero-pads 33 bins to
L=2048, so corr[s, :] is a rank-<=66 function of t; the DC term is constant
over t and cancels in softmax.  Collapsing the spectral products
(re*re + im*im -> cos row, im*re - re*im -> sin row) leaves 64 coefficient
rows: the logits are an exact K=64 matmul against a fixed cos/sin basis and
no [L, L] tensor ever exists in DRAM.

Sharding: head h -> core h (both batches per core).  Only the head-mean of
the 64 x 2048 coefficient matrix couples cores.  Default mode (SPLIT=True)
runs two NEFFs: phase A computes coefficients (~40 us), the host sums the
8 cores' 0.5 MB outputs, and phase B (~90 us) does softmax + aggregation —
this is much faster than an on-device AllReduce, which costs 55-60 us of
mostly-fixed latency on this platform (SPLIT=False keeps everything on
device in one NEFF with column-halved AllReduces, ~170 us).

Phase B details: K=64 logits matmuls are row-packed (two concurrent 64-row
PE tiles via base_partition 0/64 of duplicated coefficient/basis tensors);
the delay-aggregation matmuls are column-packed (Vt stored [128, 1024]:
partitions 0-63 hold t 0:1024, partitions 64-127 hold t 1024:2048).  The
softmax exp splits between ScalarE (table exp) and VectorE (custom DVE op
EXP8_ANT: exp(x) ~= (c0 + x(c1 + x c2))^8, valid since logits are bounded
by ~1.5), both with fused free-dim accumulation for the denominator; the
per-row 1/sum folds into the tiny v-tile instead of the weight tile.
"""
import sys
from operator import add as _op_add

sys.path.insert(0, "/opt/trn_rl_repo")

import numpy as np
import ml_dtypes

from concourse import bass, bacc, mybir, tile
from concourse import dve_ops
from concourse.dve_spec import Spec, Src0, C0, C1, C2, Zero, sq, lower
from concourse.dve_uop import DveOpSpec
from concourse.bass_utils import run_bass_kernel_spmd

B, L, E, H, D = 2, 2048, 512, 8, 64
NF = 32          # frequencies 1..32 of the 64-point rfft (DC dropped)
NCOMP = 4 * NF   # 128 raw product rows
NCC = 2 * NF     # 64 compressed coefficient rows (cos, sin)
NCORES = 8
SC = L // 128    # 16 s-chunks of 128 rows
BF16 = mybir.dt.bfloat16
F32 = mybir.dt.float32

# minimax quadratic p(z) for e^z on z = x/8, |x| <= 1.68; exp(x) ~= p(x)^8
EXP_C = (0.99970171, 0.12580122, 0.00795605)

TRACE = False
SPLIT = True
LAST_RESULT = None
LAST_RESULT_A = None

_COMPILED = None
_EXP_OP = None


def _register_exp_op():
    global _EXP_OP
    if _EXP_OP is not None:
        return _EXP_OP
    for o in dve_ops.OPS:
        if o.name == "EXP8_ANT":
            _EXP_OP = o
            return o

    body = sq(sq(sq(C0 + Src0 * (C1 + Src0 * C2))))

    def _ref(in0, in1, c0, c1, c2):
        x = in0.astype(np.float32)
        b = (((c0 + x * (c1 + x * c2)) ** 8)).astype(np.float32)
        return b, b.reshape(b.shape[0], -1).sum(axis=-1, keepdims=True)

    spec = Spec(body=body, accum=_op_add, accum_init=Zero, reference=_ref)
    opcode = dve_ops._CUSTOM_DVE_ROW_BASE + len(dve_ops.OPS)
    dve_ops._SUB_OPCODE_FOR_NAME["EXP8_ANT"] = opcode
    shas = {}
    for ver in ("v3", "v4"):
        shas[ver] = DveOpSpec(
            name="EXP8_ANT", opcode=opcode, uops=lower(spec, ver=ver), rd1_en=False
        ).sha(ver)
    op = dve_ops.DveOp("EXP8_ANT", spec, subdim=False, uops_sha=shas)
    dve_ops.OPS.append(op)
    dve_ops.CUSTOM_DVE_SPECS[op.name] = spec
    _EXP_OP = op
    return op


def _constants():
    c = np.arange(D)
    f = np.arange(1, NF + 1)
    ang = 2 * np.pi * np.outer(c, f) / D
    fcos = np.cos(ang)       # Re X_f   = sum_c q_c cos
    fsin = -np.sin(ang)      # Im X_f   = -sum_c q_c sin
    w = 2.0 / L              # irfft weight for interior bins
    fx = np.concatenate([fcos * w, fsin * w, fsin * w, fcos * w], axis=1)  # [64, 128]
    fy = np.concatenate([fcos, fsin, fcos, fsin], axis=1)                  # [64, 128]
    t = np.arange(L)
    angt = 2 * np.pi * np.outer(f, t) / L
    cosb, sinb = np.cos(angt), np.sin(angt)
    basis64 = np.concatenate([cosb, -sinb], axis=0)                        # [64, 2048]
    basisdup = np.concatenate([basis64, basis64], axis=0)                  # [128, 2048]
    # compression: Ccs[0:32] = P[0:32] + P[32:64]  (re*re + im*im -> cos)
    #              Ccs[32:64] = P[64:96] - P[96:128] (im*re - re*im -> -sin)
    mcomp = np.zeros((NCOMP, NCC), np.float32)
    for m in range(32):
        mcomp[m, m] = 1.0
        mcomp[m + 32, m] = 1.0
        mcomp[m + 64, m + 32] = 1.0
        mcomp[m + 96, m + 32] = -1.0
    bf = ml_dtypes.bfloat16
    return fx.astype(bf), fy.astype(bf), basisdup.astype(bf), mcomp.astype(bf)


def _build():
    exp_op = _register_exp_op()
    nc = bacc.Bacc("TRN2", target_bir_lowering=False, debug=False, num_devices=NCORES)

    qT_d = nc.dram_tensor("qT", [B, D, L], BF16, kind="ExternalInput")
    kT_d = nc.dram_tensor("kT", [B, D, L], BF16, kind="ExternalInput")
    v_d = nc.dram_tensor("v", [B, L, D], BF16, kind="ExternalInput")
    fx_d = nc.dram_tensor("fx", [D, NCOMP], BF16, kind="ExternalInput")
    fy_d = nc.dram_tensor("fy", [D, NCOMP], BF16, kind="ExternalInput")
    basis_d = nc.dram_tensor("basis2", [NCOMP, L], BF16, kind="ExternalInput")
    mcomp_d = nc.dram_tensor("mcomp", [NCOMP, NCC], BF16, kind="ExternalInput")
    out_d = nc.dram_tensor("out", [B, D, L], F32, kind="ExternalOutput")

    rg = [list(range(NCORES))]

    with tile.TileContext(nc) as tc:
        with (
            tc.tile_pool(name="consts", bufs=1) as consts,
            tc.tile_pool(name="qk", bufs=2) as qk_pool,
            tc.tile_pool(name="vv", bufs=2) as v_pool,
            tc.tile_pool(name="xy", bufs=2) as xy_pool,
            tc.tile_pool(name="cf", bufs=2) as cf_pool,
            tc.tile_pool(name="cs", bufs=2) as cs_pool,
            tc.tile_pool(name="cd", bufs=2) as cd_pool,
            tc.tile_pool(name="wts", bufs=6) as w_pool,
            tc.tile_pool(name="small", bufs=12) as s_pool,
            tc.tile_pool(name="outp", bufs=2) as out_pool,
            tc.tile_pool(name="ps_log", bufs=3, space="PSUM") as ps_log,
            tc.tile_pool(name="ps_vt", bufs=1, space="PSUM") as ps_vt,
            tc.tile_pool(name="dram", bufs=1, space="DRAM") as dram,
        ):
            fx_sb = consts.tile([D, NCOMP], BF16)
            fy_sb = consts.tile([D, NCOMP], BF16)
            basis_sb = consts.tile([NCOMP, L], BF16)
            mcomp_sb = consts.tile([NCOMP, NCC], BF16)
            nc.sync.dma_start(out=fx_sb[:], in_=fx_d[:])
            nc.sync.dma_start(out=fy_sb[:], in_=fy_d[:])
            nc.gpsimd.dma_start(out=basis_sb[:], in_=basis_d[:])
            nc.sync.dma_start(out=mcomp_sb[:], in_=mcomp_d[:])

            cc_in_h = [dram.tile([B * NCC, 1024], BF16, name=f"cc_in_h{j}")
                       for j in range(2)]
            cc_out_h = [dram.tile([B * NCC, 1024], BF16, addr_space="Shared",
                                  name=f"cc_out_h{j}") for j in range(2)]

            # Prefetch everything while the coefficient pipeline runs.
            qk_sb = []
            for b in range(B):
                qT_sb = qk_pool.tile([D, L], BF16, tag=f"qT{b}")
                kT_sb = qk_pool.tile([D, L], BF16, tag=f"kT{b}")
                nc.sync.dma_start(out=qT_sb[:], in_=qT_d[b])
                nc.sync.dma_start(out=kT_sb[:], in_=kT_d[b])
                qk_sb.append((qT_sb, kT_sb))
            v_sbs = []
            for b in range(B):
                v_sb = v_pool.tile([128, SC, D], BF16, tag=f"v{b}")
                nc.gpsimd.dma_start(
                    out=v_sb[:], in_=v_d[b].rearrange("(c p) d -> p c d", p=128)
                )
                v_sbs.append(v_sb)

            # ---- Phase 1: compressed coefficients Ccs, b-stacked [128, L] ----
            # Column-halved: the AllReduce for s-columns 0:1024 fires after the
            # first half of the pipeline, and its result is all that the first
            # 8 s-chunks of the main loop need — the second AllReduce hides
            # under main-loop compute.  b0 -> partitions 0:64, b1 -> 64:128.
            ccs_h = [cs_pool.tile([B * NCC, 1024], BF16, tag=f"ccs{j}", name=f"ccs_h{j}")
                     for j in range(2)]
            for j in range(2):  # s-column halves of 1024
                for b in range(B):
                    qT_sb, kT_sb = qk_sb[b]
                    xt2 = xy_pool.tile([NCOMP, 1024], BF16, tag="xt2")
                    yt2 = xy_pool.tile([NCOMP, 1024], BF16, tag="yt2")
                    engs = ("scalar", "vector") if b == 0 else ("vector", "scalar")
                    for src_sb, fmat, dst, cast_eng in (
                        (qT_sb, fx_sb, xt2, engs[0]),
                        (kT_sb, fy_sb, yt2, engs[1]),
                    ):
                        ps = ps_log.tile([NCOMP, 1024], F32, tag="log")
                        for q in range(2):
                            nc.tensor.matmul(
                                ps[:, q * 512:(q + 1) * 512],
                                fmat[:],
                                src_sb[:, j * 1024 + q * 512: j * 1024 + (q + 1) * 512],
                                start=True, stop=True,
                            )
                        if cast_eng == "scalar":
                            nc.scalar.copy(dst[:], ps[:])
                        else:
                            nc.vector.tensor_copy(dst[:], ps[:])

                    cf = cf_pool.tile([NCOMP, 1024], BF16, tag="cfull")
                    nc.vector.tensor_mul(cf[:], xt2[:], yt2[:])

                    ps = ps_log.tile([NCOMP, 1024], F32, tag="log")
                    pcc = ps[b * NCC:(b + 1) * NCC, :]
                    for q in range(2):
                        nc.tensor.matmul(
                            pcc[:, q * 512:(q + 1) * 512],
                            mcomp_sb[:],
                            cf[:, q * 512:(q + 1) * 512],
                            start=True, stop=True,
                        )
                    dst = ccs_h[j][b * NCC:(b + 1) * NCC, :]
                    nc.scalar.copy(dst, pcc)
                    nc.sync.dma_start(
                        out=cc_in_h[j][b * NCC:(b + 1) * NCC, :], in_=dst
                    )
                nc.gpsimd.collective_compute(
                    "AllReduce", mybir.AluOpType.add, replica_groups=rg,
                    ins=[cc_in_h[j][:].opt()], outs=[cc_out_h[j][:].opt()],
                )

            # cd = ccs - mean_h = (csum * -1/8) + ccs, duplicated to both
            # partition halves so K=64 logits matmuls row-pack the PE.
            cd2h = [[None, None], [None, None]]
            for j in range(2):
                csum = cs_pool.tile([B * NCC, 1024], BF16, tag=f"csum{j}",
                                    name=f"csum_h{j}")
                nc.sync.dma_start(out=csum[:], in_=cc_out_h[j][:])
                cda = cs_pool.tile([B * NCC, 1024], BF16, tag=f"cda{j}",
                                   name=f"cd_all{j}")
                nc.vector.scalar_tensor_tensor(
                    cda[:], csum[:], -1.0 / NCORES, ccs_h[j][:],
                    op0=mybir.AluOpType.mult, op1=mybir.AluOpType.add,
                )
                for b in range(B):
                    cdd = cd_pool.tile([2 * NCC, 1024], BF16, tag=f"cd2_{b}{j}",
                                       name=f"cd2_{b}{j}")
                    nc.sync.dma_start(out=cdd[0:NCC, :],
                                      in_=cda[b * NCC:(b + 1) * NCC, :])
                    nc.sync.dma_start(out=cdd[NCC:2 * NCC, :],
                                      in_=cda[b * NCC:(b + 1) * NCC, :])
                    cd2h[b][j] = cdd

            # ---- Phase 2: per-b softmax + delay aggregation ----
            # Vt packed: partitions 0-63 = Vt[:, 0:1024], 64-127 = Vt[:, 1024:2048]
            for b in range(B):
                v_sb = v_sbs[b]
                vt_ps = ps_vt.tile([128, 1024], F32, tag="vt")

                wts_hist = {}
                vts_hist = {}
                sig_hist = {}

                def emit_acc(sc):
                    pwt = wts_hist.pop(sc)
                    pvts = vts_hist.pop(sc)
                    for q in range(2):  # packed pairs: (q, q+2)
                        nc.tensor.matmul(
                            vt_ps[0:D, q * 512:(q + 1) * 512],
                            pvts[:],
                            pwt[0][:, q * 512:(q + 1) * 512],
                            start=(sc == 0), stop=(sc == SC - 1),
                        )
                        nc.tensor.matmul(
                            vt_ps[D:2 * D, q * 512:(q + 1) * 512],
                            pvts[:],
                            pwt[1][:, q * 512:(q + 1) * 512],
                            start=(sc == 0), stop=(sc == SC - 1),
                        )

                def emit_small(sc):
                    sig = sig_hist.pop(sc)
                    sigsum = s_pool.tile([128, 1], F32, tag="sigsum")
                    nc.gpsimd.tensor_add(sigsum[:], sig[:, 0:1], sig[:, 1:2])
                    rcp = s_pool.tile([128, 1], F32, tag="rcp")
                    nc.vector.reciprocal_approx_fast(rcp[:], sigsum[:])
                    vts = s_pool.tile([128, D], BF16, tag="vts")
                    nc.vector.tensor_scalar_mul(vts[:], v_sb[:, sc, :], rcp[:])
                    vts_hist[sc] = vts

                for sc in range(SC):
                    half = cd2h[b][sc // 8]
                    off = (sc % 8) * 128
                    cdt = half[0:NCC, off:off + 128]
                    cdb = half[NCC:2 * NCC, off:off + 128]
                    lg0 = ps_log.tile([128, 1024], F32, tag="log")
                    lg1 = ps_log.tile([128, 1024], F32, tag="log")
                    for q in range(2):
                        # row-packed pair: h2=0 on PE rows 0-63, h2=1 on 64-127
                        nc.tensor.matmul(
                            lg0[:, q * 512:(q + 1) * 512], cdt,
                            basis_sb[0:NCC, q * 512:(q + 1) * 512],
                            start=True, stop=True,
                        )
                        nc.tensor.matmul(
                            lg1[:, q * 512:(q + 1) * 512], cdb,
                            basis_sb[NCC:2 * NCC, 1024 + q * 512: 1024 + (q + 1) * 512],
                            start=True, stop=True,
                        )
                    if sc >= 2:
                        emit_acc(sc - 2)

                    sig = s_pool.tile([128, 2], F32, tag="sig")
                    wt0 = w_pool.tile([128, 1024], BF16, tag="wt")
                    nc.scalar.activation(
                        wt0[:], lg0[:], mybir.ActivationFunctionType.Exp,
                        accum_out=sig[:, 0:1],
                    )
                    wt1 = w_pool.tile([128, 1024], BF16, tag="wt")
                    nc.vector._custom_dve(
                        exp_op, out=wt1[:], in0=lg1[:],
                        s0=EXP_C[0], s1=EXP_C[1], imm2=EXP_C[2],
                        accum_out=sig[:, 1:2],
                    )
                    wts_hist[sc] = (wt0, wt1)
                    sig_hist[sc] = sig
                    if sc >= 1:
                        emit_small(sc - 1)

                emit_small(SC - 1)
                emit_acc(SC - 2)
                emit_acc(SC - 1)

                out_sb = out_pool.tile([128, 1024], F32, tag="out")
                nc.vector.tensor_copy(out_sb[:], vt_ps[:])
                nc.sync.dma_start(out=out_d[b][:, 0:1024], in_=out_sb[0:D, :])
                nc.sync.dma_start(out=out_d[b][:, 1024:2048], in_=out_sb[D:2 * D, :])

    nc.compile()
    return nc



_COMPILED_A = None
_COMPILED_B = None


def _build_split_a():
    """NEFF A: coefficient pipeline only.  Outputs b-stacked Ccs [128, L]."""
    _register_exp_op()
    nc = bacc.Bacc("TRN2", target_bir_lowering=False, debug=False, num_devices=NCORES)
    qk_d = nc.dram_tensor("qkT", [B, 2 * D, L], BF16, kind="ExternalInput")
    fxy_d = nc.dram_tensor("fxy", [2 * D, NCOMP], BF16, kind="ExternalInput")
    mcomp_d = nc.dram_tensor("mcomp", [NCOMP, NCC], BF16, kind="ExternalInput")
    ccs_d = nc.dram_tensor("ccs", [B * NCC, L], BF16, kind="ExternalOutput")

    with tile.TileContext(nc) as tc:
        with (
            tc.tile_pool(name="consts", bufs=1) as consts,
            tc.tile_pool(name="qk", bufs=2) as qk_pool,
            tc.tile_pool(name="xy", bufs=4) as xy_pool,
            tc.tile_pool(name="cf", bufs=4) as cf_pool,
            tc.tile_pool(name="cs", bufs=4) as cs_pool,
            tc.tile_pool(name="ps", bufs=4, space="PSUM") as ps_pool,
        ):
            fxy_sb = consts.tile([2 * D, NCOMP], BF16)
            mcomp_sb = consts.tile([NCOMP, NCC], BF16)
            nc.sync.dma_start(out=fxy_sb[:], in_=fxy_d[:])
            nc.sync.dma_start(out=mcomp_sb[:], in_=mcomp_d[:])
            qk_sb = []
            for b in range(B):
                qk_t = qk_pool.tile([2 * D, L], BF16, tag=f"qk{b}")
                nc.sync.dma_start(out=qk_t[:], in_=qk_d[b])
                qk_sb.append(qk_t)

            for b in range(B):
                qk_t = qk_sb[b]
                for j in range(2):
                    xt2 = xy_pool.tile([NCOMP, 1024], BF16, tag="xt2")
                    yt2 = xy_pool.tile([NCOMP, 1024], BF16, tag="yt2")
                    psx = ps_pool.tile([NCOMP, 1024], F32, tag="log")
                    psy = ps_pool.tile([NCOMP, 1024], F32, tag="log")
                    for q in range(2):
                        cols = slice(j * 1024 + q * 512, j * 1024 + (q + 1) * 512)
                        # row-packed pair: q-spectrum on PE rows 0-63,
                        # k-spectrum on rows 64-127, concurrent
                        nc.tensor.matmul(
                            psx[:, q * 512:(q + 1) * 512],
                            fxy_sb[0:D, :], qk_t[0:D, cols],
                            start=True, stop=True,
                        )
                        nc.tensor.matmul(
                            psy[:, q * 512:(q + 1) * 512],
                            fxy_sb[D:2 * D, :], qk_t[D:2 * D, cols],
                            start=True, stop=True,
                        )
                    eng0 = "scalar" if b == 0 else "vector"
                    if eng0 == "scalar":
                        nc.scalar.copy(xt2[:], psx[:])
                        nc.vector.tensor_copy(yt2[:], psy[:])
                    else:
                        nc.vector.tensor_copy(xt2[:], psx[:])
                        nc.scalar.copy(yt2[:], psy[:])
                    cf = cf_pool.tile([NCOMP, 1024], BF16, tag="cfull")
                    nc.vector.tensor_mul(cf[:], xt2[:], yt2[:])
                    ps = ps_pool.tile([NCOMP, 1024], F32, tag="log")
                    pcc = ps[b * NCC:(b + 1) * NCC, :]
                    for q in range(2):
                        nc.tensor.matmul(
                            pcc[:, q * 512:(q + 1) * 512],
                            mcomp_sb[:],
                            cf[:, q * 512:(q + 1) * 512],
                            start=True, stop=True,
                        )
                    ccs = cs_pool.tile([NCC, 1024], BF16, tag="ccs")
                    nc.scalar.copy(ccs[:], pcc)
                    nc.sync.dma_start(
                        out=ccs_d[b * NCC:(b + 1) * NCC, j * 1024:(j + 1) * 1024],
                        in_=ccs[:],
                    )
    nc.compile()
    return nc


def _build_split_b():
    """NEFF B: softmax + delay aggregation from host-reduced coefficients."""
    exp_op = _register_exp_op()
    nc = bacc.Bacc("TRN2", target_bir_lowering=False, debug=False, num_devices=NCORES)
    cd_d = nc.dram_tensor("cd2", [B, 2 * NCC, L], BF16, kind="ExternalInput")
    v_d = nc.dram_tensor("v", [B, L, D], BF16, kind="ExternalInput")
    basis_d = nc.dram_tensor("basis2", [NCOMP, L], BF16, kind="ExternalInput")
    out_d = nc.dram_tensor("out", [B, D, L], F32, kind="ExternalOutput")

    with tile.TileContext(nc) as tc:
        with (
            tc.tile_pool(name="consts", bufs=1) as consts,
            tc.tile_pool(name="vv", bufs=2) as v_pool,
            tc.tile_pool(name="cd", bufs=2) as cd_pool,
            tc.tile_pool(name="wts", bufs=10) as w_pool,
            tc.tile_pool(name="small", bufs=12) as s_pool,
            tc.tile_pool(name="outp", bufs=2) as out_pool,
            tc.tile_pool(name="ps_log", bufs=3, space="PSUM") as ps_log,
            tc.tile_pool(name="ps_vt", bufs=1, space="PSUM") as ps_vt,
        ):
            basis_sb = consts.tile([NCOMP, L], BF16)
            nc.sync.dma_start(out=basis_sb[:], in_=basis_d[:])
            cd_sbs = []
            v_sbs = []
            for b in range(B):
                halves = []
                for j in range(2):
                    cdd = cd_pool.tile([2 * NCC, 1024], BF16, tag=f"cd{b}{j}")
                    nc.sync.dma_start(out=cdd[:], in_=cd_d[b][:, j * 1024:(j + 1) * 1024])
                    halves.append(cdd)
                cd_sbs.append(halves)
                v_sb = v_pool.tile([128, SC, D], BF16, tag=f"v{b}")
                nc.gpsimd.dma_start(
                    out=v_sb[:], in_=v_d[b].rearrange("(c p) d -> p c d", p=128)
                )
                v_sbs.append(v_sb)

            for b in range(B):
                v_sb = v_sbs[b]
                vt_ps = ps_vt.tile([128, 1024], F32, tag="vt")
                wts_hist = {}
                vts_hist = {}
                sig_hist = {}

                def emit_acc(sc):
                    pwt = wts_hist.pop(sc)
                    pvts = vts_hist.pop(sc)
                    for q in range(2):
                        nc.tensor.matmul(
                            vt_ps[0:D, q * 512:(q + 1) * 512],
                            pvts[:],
                            pwt[0][:, q * 512:(q + 1) * 512],
                            start=(sc == 0), stop=(sc == SC - 1),
                        )
                        nc.tensor.matmul(
                            vt_ps[D:2 * D, q * 512:(q + 1) * 512],
                            pvts[:],
                            pwt[1][:, q * 512:(q + 1) * 512],
                            start=(sc == 0), stop=(sc == SC - 1),
                        )

                def emit_small(sc):
                    sig = sig_hist.pop(sc)
                    sigsum = s_pool.tile([128, 1], F32, tag="sigsum")
                    nc.gpsimd.tensor_add(sigsum[:], sig[:, 0:1], sig[:, 1:2])
                    rcp = s_pool.tile([128, 1], F32, tag="rcp")
                    nc.vector.reciprocal_approx_fast(rcp[:], sigsum[:])
                    vts = s_pool.tile([128, D], BF16, tag="vts")
                    nc.gpsimd.tensor_scalar_mul(vts[:], v_sb[:, sc, :], rcp[:])
                    vts_hist[sc] = vts

                for sc in range(SC):
                    half = cd_sbs[b][sc // 8]
                    off = (sc % 8) * 128
                    cdt = half[0:NCC, off:off + 128]
                    cdb = half[NCC:2 * NCC, off:off + 128]
                    lg0 = ps_log.tile([128, 1024], F32, tag="log")
                    lg1 = ps_log.tile([128, 1024], F32, tag="log")
                    for q in range(2):
                        nc.tensor.matmul(
                            lg0[:, q * 512:(q + 1) * 512], cdt,
                            basis_sb[0:NCC, q * 512:(q + 1) * 512],
                            start=True, stop=True,
                        )
                        nc.tensor.matmul(
                            lg1[:, q * 512:(q + 1) * 512], cdb,
                            basis_sb[NCC:2 * NCC, 1024 + q * 512: 1024 + (q + 1) * 512],
                            start=True, stop=True,
                        )
                    if sc >= 2:
                        emit_acc(sc - 2)
                    sig = s_pool.tile([128, 2], F32, tag="sig")
                    wt0 = w_pool.tile([128, 1024], BF16, tag="wt")
                    nc.scalar.activation(
                        wt0[:], lg0[:], mybir.ActivationFunctionType.Exp,
                        accum_out=sig[:, 0:1],
                    )
                    wt1 = w_pool.tile([128, 1024], BF16, tag="wt")
                    nc.vector._custom_dve(
                        exp_op, out=wt1[:], in0=lg1[:],
                        s0=EXP_C[0], s1=EXP_C[1], imm2=EXP_C[2],
                        accum_out=sig[:, 1:2],
                    )
                    wts_hist[sc] = (wt0, wt1)
                    sig_hist[sc] = sig
                    if sc >= 1:
                        emit_small(sc - 1)

                emit_small(SC - 1)
                emit_acc(SC - 2)
                emit_acc(SC - 1)

                out_sb = out_pool.tile([128, 1024], F32, tag="out")
                nc.vector.tensor_copy(out_sb[:], vt_ps[:])
                nc.sync.dma_start(out=out_d[b][:, 0:1024], in_=out_sb[0:D, :])
                nc.sync.dma_start(out=out_d[b][:, 1024:2048], in_=out_sb[D:2 * D, :])
    nc.compile()
    return nc


def _get_split():
    global _COMPILED_A, _COMPILED_B
    if _COMPILED_A is None:
        _COMPILED_A = _build_split_a()
        _COMPILED_B = _build_split_b()
    return _COMPILED_A, _COMPILED_B


def _get_compiled():
    global _COMPILED
    if _COMPILED is None:
        _COMPILED = _build()
    return _COMPILED


def kernel(queries, keys, values):
    global LAST_RESULT
    queries = np.asarray(queries, dtype=np.float32)
    keys = np.asarray(keys, dtype=np.float32)
    values = np.asarray(values, dtype=np.float32)

    fx, fy, basisdup, mcomp = _constants()
    bf = ml_dtypes.bfloat16

    in_maps = []
    for i in range(NCORES):
        sl = slice(i * D, (i + 1) * D)
        qT_i = np.ascontiguousarray(queries[:, :, sl].transpose(0, 2, 1)).astype(bf)
        kT_i = np.ascontiguousarray(keys[:, :, sl].transpose(0, 2, 1)).astype(bf)
        in_maps.append({
            "qT": qT_i,
            "kT": kT_i,
            "qkT": np.concatenate([qT_i, kT_i], axis=1),
            "v": np.ascontiguousarray(values[:, :, sl]).astype(bf),
            "fx": fx,
            "fy": fy,
            "fxy": np.concatenate([fx, fy], axis=0),
            "basis2": basisdup,
            "mcomp": mcomp,
        })

    kw = {"trace_cores": list(range(NCORES))} if TRACE else {}
    cores = list(range(NCORES))
    if SPLIT:
        nca, ncb = _get_split()
        maps_a = [{k: m[k] for k in ("qkT", "fxy", "mcomp")}
                  for m in in_maps]
        res_a = run_bass_kernel_spmd(nca, maps_a, core_ids=cores, trace=TRACE, **kw)
        ccs_all = np.stack([res_a.results[i]["ccs"] for i in range(NCORES)])
        csum = ccs_all.astype(np.float32).sum(axis=0) * (1.0 / NCORES)
        maps_b = []
        for i in range(NCORES):
            cd_all = (ccs_all[i].astype(np.float32) - csum).astype(bf)  # [128, L]
            cd2 = np.stack([np.concatenate([cd_all[b * NCC:(b + 1) * NCC]] * 2, axis=0)
                            for b in range(B)])                          # [B, 128, L]
            maps_b.append({"cd2": cd2, "v": in_maps[i]["v"],
                           "basis2": in_maps[i]["basis2"]})
        res = run_bass_kernel_spmd(ncb, maps_b, core_ids=cores, trace=TRACE, **kw)
        LAST_RESULT = res
        globals()["LAST_RESULT_A"] = res_a
    else:
        nc = _get_compiled()
        res = run_bass_kernel_spmd(nc, in_maps, core_ids=cores, trace=TRACE, **kw)
        LAST_RESULT = res

    vt_full = np.stack([res.results[i]["out"] for i in range(NCORES)], axis=1)
    # reference: out = transpose(Vt[B,H,d,L], (0,2,1,3)).reshape(B, L, H*d)
    return np.ascontiguousarray(
        vt_full.transpose(0, 2, 1, 3).reshape(B, L, E)
    ).astype(np.float32)



# revision 12
# speedup vs baseline: 2.4536x; 2.4536x over previous
"""AutoCorrelation (Autoformer-style) Bass kernel for one TRN2 chip (8 NeuronCores).

Math: per (b, h):
    corr = irfft(rfft(q, ch-axis) * conj(rfft(k, ch-axis)), n=L)   # [L, L]
    weights = softmax(corr - mean_h(corr), axis=-1)
    out = v^T @ weights                                            # [d, L]
The 64-point channel rfft zero-padded onto a 2048-point irfft makes every
corr row a 32-harmonic trig polynomial in t (frequencies 1..32 of period L),
so corr == C^T B for a 128-row coefficient matrix C (the four spectral
product blocks XcYc/XsYs/XsYc/XcYs) and a fixed cos/sin basis B.  exp of a
1.7-bounded 32-harmonic signal has negligible spectral mass beyond +-64, so
softmax + delay aggregation are evaluated on a T=128 coarse t-grid and the
tiny [d, T] output is upsampled exactly (FFT zero-pad) on the host.

Sharding: by sequence rows.  Core i gets s in [256*i, 256*(i+1)) for ALL
heads: corr rows, softmax and Z are row-local, the head-mean is core-local
(no collective!), and the s-contraction of the aggregation is completed by
summing the 8 cores' [B, H, d, T] partials on the host during the gather.
One NEFF, no AllReduce, no host round-trip between phases.

Per-core pipeline: spectra matmuls (head-pair row-packed on the PE) ->
spectral products (DVE, PSUM-direct) -> head-tree sum + per-head mean
subtract (DVE, bf16 2x) -> K=128 logits matmul per 128-row s-chunk ->
exp (ScalarE table / DVE polynomial, split) with per-row Z -> fold 1/Z
into the [128, 64] v-tile (DVE 4x tensor_scalar) -> K=128 aggregation
matmul accumulating the two s-chunks -> [d, T] f32 partials DMA'd out.
"""
import sys
from operator import add as _op_add

sys.path.insert(0, "/opt/trn_rl_repo")

import numpy as np
import ml_dtypes

from concourse import bass, bacc, mybir, tile
from concourse import dve_ops
from concourse.dve_spec import Spec, Src0, C0, C1, C2, Zero, sq, lower
from concourse.dve_uop import DveOpSpec
from concourse.bass_utils import run_bass_kernel_spmd

B, L, E, H, D = 2, 2048, 512, 8, 64
NF = 32           # frequencies 1..32 of the 64-point rfft (DC dropped)
NCOMP = 4 * NF    # 128 spectral product rows
NCORES = 8
SC = 256          # s-rows per core
T = 128           # coarse t-grid (16-sample stride); upsampled on host
BF16 = mybir.dt.bfloat16
F32 = mybir.dt.float32

# minimax quadratic p(z) for e^z on z = x/8, |x| <= 1.68; exp(x) ~= p(x)^8
EXP_C = (0.99970171, 0.12580122, 0.00795605)

TRACE = False
LAST_RESULT = None
LAST_RESULT_A = None
import os
STAGE = int(os.environ.get("K_STAGE", "9"))  # debug bisection: 9 = full kernel

_COMPILED = None
_EXP_OP = None


def _register_exp_op():
    global _EXP_OP
    if _EXP_OP is not None:
        return _EXP_OP
    for o in dve_ops.OPS:
        if o.name == "EXP8_ANT":
            _EXP_OP = o
            return o

    body = sq(sq(sq(C0 + Src0 * (C1 + Src0 * C2))))

    def _ref(in0, in1, c0, c1, c2):
        x = in0.astype(np.float32)
        b = (((c0 + x * (c1 + x * c2)) ** 8)).astype(np.float32)
        return b, b.reshape(b.shape[0], -1).sum(axis=-1, keepdims=True)

    spec = Spec(body=body, accum=_op_add, accum_init=Zero, reference=_ref)
    opcode = dve_ops._CUSTOM_DVE_ROW_BASE + len(dve_ops.OPS)
    dve_ops._SUB_OPCODE_FOR_NAME["EXP8_ANT"] = opcode
    shas = {}
    for ver in ("v3", "v4"):
        shas[ver] = DveOpSpec(
            name="EXP8_ANT", opcode=opcode, uops=lower(spec, ver=ver), rd1_en=False
        ).sha(ver)
    op = dve_ops.DveOp("EXP8_ANT", spec, subdim=False, uops_sha=shas)
    dve_ops.OPS.append(op)
    dve_ops.CUSTOM_DVE_SPECS[op.name] = spec
    _EXP_OP = op
    return op


def _constants():
    c = np.arange(D)
    f = np.arange(1, NF + 1)
    ang = 2 * np.pi * np.outer(c, f) / D
    fcos = np.cos(ang)        # Re X_f   = sum_c q_c cos
    fsin = -np.sin(ang)       # Im X_f   = -sum_c q_c sin
    w = 2.0 / L               # irfft weight for interior bins
    fx = np.concatenate([fcos * w, fsin * w, fsin * w, fcos * w], axis=1)  # [64,128]
    fy = np.concatenate([fcos, fsin, fcos, fsin], axis=1)                  # [64,128]
    fx2 = np.concatenate([fx, fx], axis=0)   # [128, 128] head-pair packed
    fy2 = np.concatenate([fy, fy], axis=0)
    tau = np.arange(T) * (L // T)
    angt = 2 * np.pi * np.outer(f, tau) / L
    cosb, sinb = np.cos(angt), np.sin(angt)
    # product rows [wXcYc; wXsYs; wXsYc; wXcYs] pair with [cos; cos; -sin; sin]
    basis = np.concatenate([cosb, cosb, -sinb, sinb], axis=0)  # [128, T]
    bf = ml_dtypes.bfloat16
    return fx2.astype(bf), fy2.astype(bf), basis.astype(bf)


def _build():
    exp_op = _register_exp_op()
    nc = bacc.Bacc("TRN2", target_bir_lowering=False, debug=False, num_devices=NCORES)

    qk_d = nc.dram_tensor("qk", [B, 4, 2, 128, SC], BF16, kind="ExternalInput")
    v_d = nc.dram_tensor("v", [B, 2, 128, E], BF16, kind="ExternalInput")
    fx2_d = nc.dram_tensor("fx2", [128, NCOMP], BF16, kind="ExternalInput")
    fy2_d = nc.dram_tensor("fy2", [128, NCOMP], BF16, kind="ExternalInput")
    basis_d = nc.dram_tensor("basis", [NCOMP, T], BF16, kind="ExternalInput")
    out_d = nc.dram_tensor("out", [B, 4, 128, T], F32, kind="ExternalOutput")

    with tile.TileContext(nc) as tc:
        with (
            tc.tile_pool(name="consts", bufs=1) as consts,
            tc.tile_pool(name="qk", bufs=1) as qk_pool,
            tc.tile_pool(name="pp", bufs=1) as p_pool,
            tc.tile_pool(name="yy", bufs=4) as y_pool,
            tc.tile_pool(name="wts", bufs=6) as w_pool,
            tc.tile_pool(name="small", bufs=8) as s_pool,
            tc.tile_pool(name="outp", bufs=4) as out_pool,
            tc.tile_pool(name="ps_xy", bufs=4, space="PSUM") as ps_xy,
            tc.tile_pool(name="ps_lg", bufs=2, space="PSUM") as ps_lg,
            tc.tile_pool(name="ps_out", bufs=2, space="PSUM") as ps_out,
        ):
            fx2_sb = consts.tile([128, NCOMP], BF16)
            fy2_sb = consts.tile([128, NCOMP], BF16)
            basis_sb = consts.tile([NCOMP, T], BF16)
            nc.sync.dma_start(out=fx2_sb[:], in_=fx2_d[:])
            nc.sync.dma_start(out=fy2_sb[:], in_=fy2_d[:])
            nc.sync.dma_start(out=basis_sb[:], in_=basis_d[:])

            qk_sb = qk_pool.tile([128, B, 4, 2, SC], BF16, tag="qk")
            for b in range(B):
                nc.sync.dma_start(
                    out=qk_sb[:, b], in_=qk_d[b].rearrange("g q p s -> p g q s")
                )
            v_sb = qk_pool.tile([128, B, 2, E], BF16, tag="v")
            nc.gpsimd.dma_start(
                out=v_sb[:], in_=v_d.rearrange("b c p e -> p b c e")
            )

            # persistent per-b tensors
            P_sb = [p_pool.tile([128, H, SC], BF16, tag=f"P{b}", name=f"P{b}")
                    for b in range(B)]
            cd_sb = [p_pool.tile([128, H, SC], BF16, tag=f"cd{b}", name=f"cd{b}")
                     for b in range(B)]
            acc4 = [p_pool.tile([128, 4, SC], BF16, tag=f"a4{b}", name=f"a4{b}")
                    for b in range(B)]
            acc2 = [p_pool.tile([128, 2, SC], BF16, tag=f"a2{b}", name=f"a2{b}")
                    for b in range(B)]
            sumP = [p_pool.tile([128, SC], BF16, tag=f"sp{b}", name=f"sp{b}")
                    for b in range(B)]
            sig = s_pool.tile([128, B, H, 2], F32, tag="sig", name="sig")
            rcp = s_pool.tile([128, B, H, 2], F32, tag="rcp", name="rcp")

            def coef(b, hp):
                """Spectra + products for head pair (2hp, 2hp+1) of batch b."""
                px = [ps_xy.tile([NCOMP, SC], F32, tag="ps", name=f"px{j}")
                      for j in range(2)]
                py = [ps_xy.tile([NCOMP, SC], F32, tag="ps", name=f"py{j}")
                      for j in range(2)]
                for h2 in range(2):
                    rows = slice(h2 * D, (h2 + 1) * D)
                    nc.tensor.matmul(px[h2][:], fx2_sb[rows, :],
                                     qk_sb[rows, b, hp, 0, :],
                                     start=True, stop=True)
                for h2 in range(2):
                    rows = slice(h2 * D, (h2 + 1) * D)
                    nc.tensor.matmul(py[h2][:], fy2_sb[rows, :],
                                     qk_sb[rows, b, hp, 1, :],
                                     start=True, stop=True)
                ysb = y_pool.tile([NCOMP, 2, SC], BF16, tag="ysb")
                for h2 in range(2):
                    nc.scalar.copy(ysb[:, h2, :], py[h2][:])
                for h2 in range(2):
                    h = 2 * hp + h2
                    nc.vector.tensor_mul(P_sb[b][:, h, :], px[h2][:],
                                         ysb[:, h2, :])

            def tree(b):
                nc.vector.tensor_add(acc4[b][:], P_sb[b][:, 0:4, :],
                                     P_sb[b][:, 4:8, :])
                nc.vector.tensor_add(acc2[b][:], acc4[b][:, 0:2, :],
                                     acc4[b][:, 2:4, :])
                nc.vector.tensor_add(sumP[b][:], acc2[b][:, 0, :],
                                     acc2[b][:, 1, :])
                for h in range(H):
                    nc.vector.scalar_tensor_tensor(
                        cd_sb[b][:, h, :], sumP[b][:], -1.0 / H, P_sb[b][:, h, :],
                        op0=mybir.AluOpType.mult, op1=mybir.AluOpType.add,
                    )

            out_ps = {}

            def softmax_head(b, h):
                lg = [ps_lg.tile([128, T], F32, tag="lg", name=f"lg{j}")
                      for j in range(2)]
                for c in range(2):
                    nc.tensor.matmul(
                        lg[c][:],
                        cd_sb[b][:, h, c * 128:(c + 1) * 128],
                        basis_sb[:],
                        start=True, stop=True,
                    )
                wt = w_pool.tile([128, 2, T], BF16, tag="wt")
                if h % 2 == 0 or STAGE == 6:
                    for c in range(2):
                        nc.scalar.activation(
                            wt[:, c, :], lg[c][:],
                            mybir.ActivationFunctionType.Exp,
                        )
                    nc.vector.tensor_reduce(
                        sig[:, b, h, :], wt[:],
                        axis=mybir.AxisListType.X, op=mybir.AluOpType.add,
                    )
                else:
                    for c in range(2):
                        nc.vector._custom_dve(
                            exp_op, out=wt[:, c, :], in0=lg[c][:],
                            s0=EXP_C[0], s1=EXP_C[1], imm2=EXP_C[2],
                            accum_out=sig[:, b, h, c:c + 1],
                        )
                nc.vector.reciprocal_approx_fast(rcp[:, b, h, :], sig[:, b, h, :])
                hp, h2 = h // 2, h % 2
                if h2 == 0:
                    out_ps[(b, hp)] = ps_out.tile([128, T], F32, tag="po",
                                                  name=f"po{b}{hp}")
                po = out_ps[(b, hp)]
                for c in range(2):
                    vts = s_pool.tile([128, D], BF16, tag="vts")
                    nc.vector.tensor_scalar_mul(
                        vts[:], v_sb[:, b, c, h * D:(h + 1) * D],
                        rcp[:, b, h, c:c + 1],
                    )
                    nc.tensor.matmul(
                        po[h2 * D:(h2 + 1) * D, :], vts[:], wt[:, c, :],
                        start=(c == 0), stop=(c == 1),
                    )
                if h2 == 1:
                    po = out_ps.pop((b, hp))
                    ob = out_pool.tile([128, T], F32, tag="ob")
                    nc.scalar.copy(ob[:], po[:])
                    nc.sync.dma_start(out=out_d[b, hp], in_=ob[:])

            def dummy_out():
                for b in range(B):
                    for hp in range(4):
                        ob = out_pool.tile([128, T], F32, tag="ob")
                        nc.vector.tensor_copy(ob[:], qk_sb[:, b, hp, 0, 0:T])
                        nc.sync.dma_start(out=out_d[b, hp], in_=ob[:])

            if STAGE <= 1:
                # stage 1: DMAs in + dummy out only
                dummy_out()
            elif STAGE <= 4:
                # stage 2: + spectra/products; 3: (same w/ scalar copy);
                # 4: + tree/cdsub
                for b in range(B):
                    for hp in range(4):
                        coef(b, hp)
                    if STAGE >= 4:
                        tree(b)
                dummy_out()
            else:
                # stage 5: scalar-exp only via STAGE==6 switch trick is below
                # b0 coefficients
                for hp in range(4):
                    coef(0, hp)
                tree(0)
                # interleave b0 softmax with b1 coefficients
                for hp in range(4):
                    softmax_head(0, 2 * hp)
                    coef(1, hp)
                    softmax_head(0, 2 * hp + 1)
                tree(1)
                for h in range(H):
                    softmax_head(1, h)

    nc.compile()
    return nc


def _get_compiled():
    global _COMPILED
    if _COMPILED is None:
        _COMPILED = _build()
    return _COMPILED


def kernel(queries, keys, values):
    global LAST_RESULT
    queries = np.asarray(queries, dtype=np.float32)
    keys = np.asarray(keys, dtype=np.float32)
    values = np.asarray(values, dtype=np.float32)

    fx2, fy2, basis = _constants()
    bf = ml_dtypes.bfloat16

    in_maps = []
    for i in range(NCORES):
        sl = slice(i * SC, (i + 1) * SC)
        qT = np.ascontiguousarray(
            queries[:, sl, :].transpose(0, 2, 1)).reshape(B, 4, 128, SC)
        kT = np.ascontiguousarray(
            keys[:, sl, :].transpose(0, 2, 1)).reshape(B, 4, 128, SC)
        qk = np.stack([qT, kT], axis=2)              # [B, 4, 2, 128, SC]
        v = values[:, sl, :].reshape(B, 2, 128, E)
        in_maps.append({
            "qk": qk.astype(bf),
            "v": np.ascontiguousarray(v).astype(bf),
            "fx2": fx2,
            "fy2": fy2,
            "basis": basis,
        })

    kw = {"trace_cores": list(range(NCORES))} if TRACE else {}
    nc = _get_compiled()
    res = run_bass_kernel_spmd(nc, in_maps, core_ids=list(range(NCORES)),
                               trace=TRACE, **kw)
    LAST_RESULT = res

    # sum the s-partials over cores: [B, 4, 128, T] -> [B, H, D, T]
    acc = np.zeros((B, 4, 128, T), np.float32)
    for i in range(NCORES):
        acc += res.results[i]["out"]
    outc = acc.reshape(B, H, D, T)
    # exact FFT upsample T -> L;  the (L/T) interp gain cancels the coarse-Z
    # underestimate, so no scale factor
    F = np.fft.rfft(outc, axis=-1)
    Ff = np.zeros((B, H, D, L // 2 + 1), complex)
    Ff[..., :T // 2 + 1] = F
    Ff[..., T // 2] *= 0.5
    vt_full = np.fft.irfft(Ff, n=L, axis=-1)
    # reference quirk: out = transpose(Vt[B,H,d,L], (0,2,1,3)).reshape(B, L, E)
    return np.ascontiguousarray(
        vt_full.transpose(0, 2, 1, 3).reshape(B, L, E)
    ).astype(np.float32)
